# revision 54
# baseline (speedup 1.0000x reference)
"""Multi-head causal attention (B=2, T=2048, C=1024, H=16, HS=64) on 8 TRN2
NeuronCores.

Sharding: 2 heads per core (tensor parallel). Each core receives the full
(pre-transposed) activations xT [B, C, T] in bf16, its 2 heads' QKV weight
slices packed [128, 8, 128] (DMA-line-friendly layout), and its 128-column
slice of w_proj transposed [128, C]. Each core computes a partial output
[B, T, C] in bf16; the host sums the 8 partials in f64 and adds b_proj.

Per-core kernel (all matmul operands bf16 -> FWL weight loads, half the
DMA bytes of f32, and less PE power so the SW throttler engages later):
  - QT/KT/VT [128(2 heads x 64), T] via lhsT=weight chunks, rhs=xT chunks.
  - V_aug [keys, 128]: V (cols 0:64, one full-width PE-transpose of VT per
    key block covers both heads) | ones via memset (64:128).
  - Flash-style causal attention in transposed layout: S^T[keys, q] blocks
    with BOTH heads in one per-j PSUM tile -- h0 in rows 0:64 / bank A,
    h1 in rows 64:128 / bank B -- so the two K=64 matmuls issue adjacent
    and run CONCURRENTLY via PE row-group tiling. exp on ScalarE (no max
    subtraction -- scores are O(1) by construction), one call per j
    covering both heads; triangular masking on GpSimd. O^T = [V|1].T @ P^T
    accumulated over key blocks gives O rows (0:64) and the softmax sums l
    (rows 64:128) in one pass.
  - Normalize straight out of PSUM: packed reciprocal_approx_fast, then
    per-head tensor_tensor reading O rows from PSUM.
  - Output projection: lhsT = OhatT t-chunks, rhs = w_projT slice, emitted
    as per-128-row chunks used as PE filler inside the attention jg loops
    (between the S^T pair and the exp-dependent O^T, covering exp latency).

Schedule: batch 1's QKV interleaves batch 0's attention groups; batch 1's
attention groups run in REVERSE (g3 first) so the drain phase ends with
the lightest group plus pure-PE proj chunks. DMA rides three hardware
queues (queue = issuing engine): scalar carries weights, sync/gpsimd split
the xt tiles at the head and the 8MB of bf16 output writes in the second
half -- inputs always ahead of outputs on their queue.
"""

import math
import sys
from contextlib import ExitStack

if "/opt/trn_rl_repo" not in sys.path:
    sys.path.insert(0, "/opt/trn_rl_repo")

import numpy as np

import concourse.mybir as mybir
import concourse.tile as tile
from concourse import bacc
from concourse.bass import ts
from concourse.bass_utils import run_bass_kernel_spmd
from concourse.tile_rust import add_dep_helper

B, T, C = 2, 2048, 1024
H, HS = 16, 64
NCORES = 8
HPC = H // NCORES  # heads per core
P = 128
G = 512  # q-group size
NG = T // G
KB = 128  # key block
NPO = C // P  # contraction chunks
F32 = mybir.dt.float32
F32R = mybir.dt.float32r
BF16 = mybir.dt.bfloat16

_nc_cache = {}


def _emit(tc):
    nc = tc.nc
    _last_pe = [None]
    xt = nc.dram_tensor("xt", [B, C, T], BF16, kind="ExternalInput").ap()
    wq2 = nc.dram_tensor("wq2", [P, NPO, 128], BF16, kind="ExternalInput").ap()
    wk2 = nc.dram_tensor("wk2", [P, NPO, 128], BF16, kind="ExternalInput").ap()
    wv2 = nc.dram_tensor("wv2", [P, NPO, 128], BF16, kind="ExternalInput").ap()
    wpt = nc.dram_tensor("wpt", [128, C], BF16, kind="ExternalInput").ap()
    tri = nc.dram_tensor("tri", [P, P], BF16, kind="ExternalInput").ap()
    identd = nc.dram_tensor("ident", [P, P], BF16, kind="ExternalInput").ap()
    out = nc.dram_tensor("out", [B, T, C], BF16, kind="ExternalOutput").ap()

    ctx = ExitStack()
    persist = ctx.enter_context(tc.tile_pool(name="persist", bufs=1))
    xt_pool = ctx.enter_context(tc.tile_pool(name="xtp", bufs=8))
    qk_pool = ctx.enter_context(tc.tile_pool(name="qkp", bufs=2))
    vt_pool = ctx.enter_context(tc.tile_pool(name="vtp", bufs=2))
    vaug_pool = ctx.enter_context(tc.tile_pool(name="vaugp", bufs=2))
    pt_pool = ctx.enter_context(tc.tile_pool(name="ptp", bufs=4))
    norm_pool = ctx.enter_context(tc.tile_pool(name="normp", bufs=2))
    ohat_pool = ctx.enter_context(tc.tile_pool(name="ohatp", bufs=2))
    out_pool = ctx.enter_context(tc.tile_pool(name="outp", bufs=6))
    st_psum = ctx.enter_context(tc.tile_pool(name="stps", bufs=1, space="PSUM"))
    ot_psum = ctx.enter_context(tc.tile_pool(name="otps", bufs=2, space="PSUM"))
    mm_psum = ctx.enter_context(tc.tile_pool(name="mmps", bufs=2, space="PSUM"))

    wq_sb = persist.tile([P, NPO, 128], BF16, tag="wq")
    wk_sb = persist.tile([P, NPO, 128], BF16, tag="wk")
    wv_sb = persist.tile([P, NPO, 128], BF16, tag="wv")
    wpt_sb = persist.tile([P, C], BF16, tag="wpt")
    tri_sb = persist.tile([P, P], BF16, tag="tri")
    ident = persist.tile([P, P], BF16, tag="ident")

    # ---- input loading: per-tg xT tiles, one 2MB DMA each ----
    def load_xt_tg(eng, b, tg, dep=None):
        t = xt_pool.tile([P, NPO, 512], BF16, tag="xt", name=f"xt{b}{tg}")
        i = eng.dma_start(
            t[:],
            xt[b, :, ts(tg, 512)].rearrange("(po pi) t -> pi po t", pi=P),
        )
        if dep is not None:
            add_dep_helper(i.ins, dep.ins, sync=True)
        return t, i

    # Fast ramp: tile0's po-chunks split across sync+scalar queues, with the
    # weight loads interleaved on scalar in the order the QKV groups need
    # them (wq before the first matmul, wk/wv before the K/V groups).
    # three queues in parallel at the head: scalar = weights only (wk/wv
    # land before the K/V groups need them), sync = even po-chunks + later
    # b0 tiles, gpsimd = odd po-chunks (idle until the b1 prefetch anyway)
    # wq in two pieces: the QKV po-chain's first matmuls start as soon as
    # the first two weight chunks land
    nc.scalar.dma_start(wq_sb[:, 0:2, :], wq2[:, 0:2, :])
    nc.scalar.dma_start(wq_sb[:, 2:, :], wq2[:, 2:, :])
    t00 = xt_pool.tile([P, NPO, 512], BF16, tag="xt", name="xt00")
    t00_last = None
    for po in range(NPO):
        eng = nc.sync if po % 2 == 0 else nc.gpsimd
        t00_last = eng.dma_start(t00[:, po, :], xt[0, ts(po, P), 0:512])
    nc.scalar.dma_start(wk_sb[:], wk2[:])
    nc.scalar.dma_start(wv_sb[:], wv2[:])
    nc.scalar.dma_start(tri_sb[:], tri[:])
    nc.scalar.dma_start(ident[:], identd[:])
    xt0, xt0_dmas = [t00], [t00_last]
    for tg in range(1, NG):
        eng = nc.sync if tg != 2 else nc.scalar
        t, i = load_xt_tg(eng, 0, tg, dep=xt0_dmas[tg - 2] if tg >= 2 else None)
        xt0.append(t)
        xt0_dmas.append(i)
        if tg == 1:
            nc.scalar.dma_start(wpt_sb[:], wpt[:])

    def new_state(b, xts):
        st = {
            "b": b,
            "xt": xts,
            "qt": qk_pool.tile([P, T], BF16, tag="qt", name=f"qt{b}"),
            "kt": qk_pool.tile([P, T], BF16, tag="kt", name=f"kt{b}"),
            "vt": vt_pool.tile([P, T], BF16, tag="vt", name=f"vt{b}"),
            "ohat": ohat_pool.tile([P, T], BF16, tag="ohat", name=f"oh{b}"),
            "vaug": [],
        }
        for h in range(HPC):
            va = vaug_pool.tile(
                [P, T // KB, 128], BF16, tag=f"vaug{h}", name=f"va{b}{h}"
            )
            nc.vector.memset(va[:, :, 64:128], 1.0)
            st["vaug"].append(va)
        return st

    def emit_qkv_group(st, which, tg):
        w_sb, dst = {
            "q": (wq_sb, st["qt"]),
            "k": (wk_sb, st["kt"]),
            "v": (wv_sb, st["vt"]),
        }[which]
        ps = mm_psum.tile([P, 512], F32, tag="mm", name=f"qkv{which}{tg}")
        for po in range(NPO):
            mi = nc.tensor.matmul(
                ps[:],
                w_sb[:, po, :],
                st["xt"][tg][:, po, :],
                start=(po == 0),
                stop=(po == NPO - 1),
            )
            _last_pe[0] = mi
        nc.vector.tensor_copy(dst[:, ts(tg, 512)], ps[:])

    def emit_vaug_part(st, kbg):
        vaug = st["vaug"]
        tps = mm_psum.tile([P, 4, P], BF16, tag="mm", name=f"vtr{kbg}")
        for kk in range(4):
            kb = 4 * kbg + kk
            nc.tensor.transpose(tps[:, kk, :], st["vt"][:, ts(kb, KB)], ident[:])
        for h in range(HPC):
            nc.vector.tensor_copy(
                vaug[h][:, 4 * kbg : 4 * kbg + 4, 0:64],
                tps[:, :, 64 * h : 64 * h + 64],
            )

    def emit_attn_g(st, g, filler=None):
        b, qt, kt, vaug, ohat = st["b"], st["qt"], st["kt"], st["vaug"], st["ohat"]
        l_sb = norm_pool.tile([P, G], F32, tag="lsb", name=f"l{b}{g}")
        rinv = norm_pool.tile([P, G], F32, tag="rinv", name=f"r{b}{g}")
        otps_h = [
            ot_psum.tile([P, G], F32, tag="ot", name=f"ot{h}") for h in range(HPC)
        ]
        n_j = 4 * g + 4
        for jg in range(math.ceil(n_j / 2)):
            js = [j for j in (2 * jg, 2 * jg + 1) if j < n_j]
            diag = 2 * jg >= 4 * g
            # ONE 4-bank tile per jg holding (j0h0, j0h1, j1h0, j1h1): each
            # j's head pair issues back-to-back with disjoint PE row groups
            # and banks (concurrent), and exp becomes a single call per jg,
            # amortizing ScalarE's 352-cycle per-call overhead
            stps = st_psum.tile([P, 4, G], F32, tag="st", name="st")
            pt = pt_pool.tile([P, 4, G], BF16, tag="pt", name="pt")
            for idx, j in enumerate(js):
                r = j - 4 * g
                q0 = 128 * r if r >= 0 else 0
                for h in range(HPC):
                    hb = 64 * h
                    nc.tensor.matmul(
                        stps[:, 2 * idx + h, q0:G],
                        kt[hb : hb + 64, ts(j, KB)],
                        qt[hb : hb + 64, G * g + q0 : G * (g + 1)],
                        start=True,
                        stop=True,
                    )
            # (the later j's cols q00:q00+128 are unread garbage on diag)
            q00 = 128 * (js[0] - 4 * g) if diag else 0
            nc.scalar.activation(
                pt[:, :, q00:G],
                stps[:, :, q00:G],
                mybir.ActivationFunctionType.Exp,
                scale=float(HS) ** -0.5,
            )
            if diag:
                for idx, j in enumerate(js):
                    q0 = 128 * (j - 4 * g)
                    for h in range(HPC):
                        nc.gpsimd.tensor_tensor(
                            pt[:, 2 * idx + h, q0 : q0 + 128],
                            pt[:, 2 * idx + h, q0 : q0 + 128],
                            tri_sb[:],
                            mybir.AluOpType.mult,
                        )
            # independent filler work sits BETWEEN the S^T pair and the
            # exp-dependent O^T in the PE queue, covering the exp latency
            if filler is not None:
                filler()
            for idx, j in enumerate(js):
                r = j - 4 * g
                q0 = 128 * r if r >= 0 else 0
                for h in range(HPC):
                    nc.tensor.matmul(
                        otps_h[h][:, q0:G],
                        vaug[h][:, j, :],
                        pt[:, 2 * idx + h, q0:G],
                        start=(j == 0),
                        stop=(j == n_j - 1),
                    )
        # normalize straight out of PSUM: l rows copied (for a packed 128-wide
        # reciprocal), O rows consumed by tensor_tensor directly from PSUM
        for h in range(HPC):
            hb = 64 * h
            nc.vector.tensor_copy(l_sb[hb : hb + 64, :], otps_h[h][64:128, :])
        nc.vector.reciprocal_approx_fast(rinv[:], l_sb[:])
        for h in range(HPC):
            hb = 64 * h
            nc.vector.tensor_tensor(
                ohat[hb : hb + 64, ts(g, G)],
                otps_h[h][0:64, :],
                rinv[hb : hb + 64, :],
                mybir.AluOpType.mult,
            )

    _chunk_ctr = [0]

    def emit_proj_chunk(st, g, tc4, scalar_ok=True, dma_eng=None):
        b, ohat = st["b"], st["ohat"]
        t0 = G * g + P * tc4
        o_sb = out_pool.tile([P, C], BF16, tag="osb", name=f"osb{b}{g}{tc4}")
        for n in range(C // 512):
            pj = mm_psum.tile([P, 512], F32, tag="mm", name=f"pj{n}")
            nc.tensor.matmul(
                pj[:],
                ohat[:, t0 : t0 + P],
                wpt_sb[:, ts(n, 512)],
                start=True,
                stop=True,
            )
            # ~3/8 of the PSUM->SBUF copies go to ScalarE (except in the
            # scalar-bound drain phase), the rest to VectorE
            if scalar_ok and (2 * tc4 + n) % 8 in (1, 4, 6):
                nc.scalar.copy(o_sb[:, ts(n, 512)], pj[:])
            else:
                nc.vector.tensor_copy(o_sb[:, ts(n, 512)], pj[:])
        # alternate the two idle hardware queues (sync/gpsimd) for the 8MB
        # of output writes; per-queue FIFO keeps them behind the xt
        # prefetches, so write throughput doubles without starving inputs
        eng = dma_eng
        if eng is None:
            eng = nc.sync if _chunk_ctr[0] % 2 == 0 else nc.gpsimd
            _chunk_ctr[0] += 1
        eng.dma_start(out[b, t0 : t0 + P, :], o_sb[:])

    def emit_proj(st, g):
        for tc4 in range(G // P):
            emit_proj_chunk(st, g, tc4)

    # ================= pipelined emission =================
    st0 = new_state(0, xt0)
    st1 = new_state(1, [None] * NG)

    pending = []
    scalar_ok = [True]

    def filler():
        if pending:
            emit_proj_chunk(*pending.pop(0), scalar_ok=scalar_ok[0])

    # b1's attention groups g1/g2 interleave INTO the b0 loop as soon as
    # their QKV tiles exist, smoothing the scalar:PE ratio globally instead
    # of a PE-bound first half and a scalar-paced second half. b1's g0 (the
    # lightest group) runs last so the drain is cheap; proj chunks fill the
    # exp-latency slots of every late attention group.
    scalar_ok[0] = False
    xt1_dmas = []
    for tg in range(NG):
        for which in ("q", "k", "v"):
            emit_qkv_group(st0, which, tg)
        emit_vaug_part(st0, tg)
        emit_attn_g(st0, tg, filler=filler if tg == 3 else None)
        pending.extend((st0, tg, tc4) for tc4 in range(G // P))
        # b1 tiles on gpsimd's own hardware queue, gated behind b0's tiles
        # so they don't steal fabric bandwidth while b0 still needs it
        t, i = load_xt_tg(
            nc.gpsimd, 1, tg,
            dep=xt1_dmas[tg - 2] if tg >= 2 else xt0_dmas[NG - 1],
        )
        st1["xt"][tg] = t
        xt1_dmas.append(i)
        if tg >= 1:
            for which in ("q", "k", "v"):
                emit_qkv_group(st1, which, tg - 1)
        if tg == 2:
            emit_vaug_part(st1, 0)
            emit_vaug_part(st1, 1)
            emit_attn_g(st1, 1, filler=filler)
            pending.extend((st1, 1, tc4) for tc4 in range(G // P))
        if tg == 3:
            emit_vaug_part(st1, 2)
            emit_attn_g(st1, 2, filler=filler)
            pending.extend((st1, 2, tc4) for tc4 in range(G // P))
    for which in ("q", "k", "v"):
        emit_qkv_group(st1, which, NG - 1)
    emit_vaug_part(st1, 3)
    emit_attn_g(st1, 3, filler=filler)
    pending.extend((st1, 3, tc4) for tc4 in range(G // P))
    emit_attn_g(st1, 0, filler=lambda: (filler(), filler()))
    pending.extend((st1, 0, tc4) for tc4 in range(G // P))
    # final flush: ScalarE's exp work is done, so its DMA queue joins the
    # rotation -- three queues drain the trailing output writes
    qrot = [nc.sync, nc.gpsimd, nc.scalar]
    qi = 0
    while pending:
        emit_proj_chunk(*pending.pop(0), scalar_ok=False, dma_eng=qrot[qi % 3])
        qi += 1
    ctx.close()


def _build():
    if "nc" in _nc_cache:
        return _nc_cache["nc"]
    nc = bacc.Bacc("TRN2", target_bir_lowering=False, debug=False)
    with tile.TileContext(nc) as tc:
        _emit(tc)
    nc.compile()
    _nc_cache["nc"] = nc
    return nc


def _make_in_maps(x, wq, wk, wv, w_proj):
    import ml_dtypes

    bf16 = ml_dtypes.bfloat16
    xt = np.ascontiguousarray(x.transpose(0, 2, 1)).astype(bf16)
    tri = np.triu(np.ones((P, P), dtype=np.float32)).astype(bf16)
    ident = np.eye(P, dtype=np.float32).astype(bf16)
    def pack_w(w2):
        # [C, 128] -> [pi, po, d] so each partition's SBUF row is one
        # contiguous 2KB DMA line
        return np.ascontiguousarray(
            w2.reshape(NPO, P, 128).transpose(1, 0, 2)
        ).astype(bf16)

    in_maps = []
    for c in range(NCORES):
        h0 = HPC * c
        in_maps.append(
            {
                "xt": xt,
                "wq2": pack_w(
                    np.concatenate([wq[h0 + i] for i in range(HPC)], axis=1)
                ),
                "wk2": pack_w(
                    np.concatenate([wk[h0 + i] for i in range(HPC)], axis=1)
                ),
                "wv2": pack_w(
                    np.concatenate([wv[h0 + i] for i in range(HPC)], axis=1)
                ),
                "wpt": np.ascontiguousarray(
                    w_proj[:, 128 * c : 128 * (c + 1)].T
                ).astype(bf16),
                "tri": tri,
                "ident": ident,
            }
        )
    return in_maps


def kernel(x, wq, wk, wv, w_proj, b_proj):
    x = np.asarray(x, dtype=np.float32)
    wq = np.asarray(wq, dtype=np.float32)
    wk = np.asarray(wk, dtype=np.float32)
    wv = np.asarray(wv, dtype=np.float32)
    w_proj = np.asarray(w_proj, dtype=np.float32)
    b_proj = np.asarray(b_proj, dtype=np.float32)

    nc = _build()
    in_maps = _make_in_maps(x, wq, wk, wv, w_proj)
    res = run_bass_kernel_spmd(nc, in_maps, core_ids=list(range(NCORES)))
    acc = np.zeros((B, T, C), dtype=np.float64)
    for r in res.results:
        acc += np.asarray(r["out"], dtype=np.float32)
    return (acc + b_proj).astype(np.float32)



# revision 55
# speedup vs baseline: 1.1866x; 1.1866x over previous
"""Multi-head causal attention (B=2, T=2048, C=1024, H=16, HS=64) on 8 TRN2
NeuronCores.

Sharding: 2 heads per core (tensor parallel). Each core receives the full
(pre-transposed) activations xT [B, C, T] in bf16, its 2 heads' QKV weight
slices packed [128, 8, 128] (DMA-line-friendly layout), and its 128-column
slice of w_proj transposed [128, C]. Each core computes a partial output
[B, T, C] in bf16; the host sums the 8 partials in f64 and adds b_proj.

Per-core kernel (all matmul operands bf16 -> FWL weight loads, half the
DMA bytes of f32, and less PE power so the SW throttler engages later):
  - QT/KT/VT [128(2 heads x 64), T] via lhsT=weight chunks, rhs=xT chunks.
  - V_aug [keys, 128]: V (cols 0:64, one full-width PE-transpose of VT per
    key block covers both heads) | ones via memset (64:128).
  - Flash-style causal attention in transposed layout: S^T[keys, q] blocks
    with BOTH heads in one per-j PSUM tile -- h0 in rows 0:64 / bank A,
    h1 in rows 64:128 / bank B -- so the two K=64 matmuls issue adjacent
    and run CONCURRENTLY via PE row-group tiling. exp on ScalarE (no max
    subtraction -- scores are O(1) by construction), one call per j
    covering both heads; triangular masking on GpSimd. O^T = [V|1].T @ P^T
    accumulated over key blocks gives O rows (0:64) and the softmax sums l
    (rows 64:128) in one pass.
  - Normalize straight out of PSUM: packed reciprocal_approx_fast, then
    per-head tensor_tensor reading O rows from PSUM.
  - Output projection: lhsT = OhatT t-chunks, rhs = w_projT slice, emitted
    as per-128-row chunks used as PE filler inside the attention jg loops
    (between the S^T pair and the exp-dependent O^T, covering exp latency).

Schedule: batch 1's QKV interleaves batch 0's attention groups; batch 1's
attention groups run in REVERSE (g3 first) so the drain phase ends with
the lightest group plus pure-PE proj chunks. DMA rides three hardware
queues (queue = issuing engine): scalar carries weights, sync/gpsimd split
the xt tiles at the head and the 8MB of bf16 output writes in the second
half -- inputs always ahead of outputs on their queue.
"""

import math
import sys
from contextlib import ExitStack

if "/opt/trn_rl_repo" not in sys.path:
    sys.path.insert(0, "/opt/trn_rl_repo")

import numpy as np

import concourse.mybir as mybir
import concourse.tile as tile
from concourse import bacc
from concourse.bass import ts
from concourse.bass_utils import run_bass_kernel_spmd
from concourse.tile_rust import add_dep_helper

B, T, C = 2, 2048, 1024
H, HS = 16, 64
NCORES = 8
HPC = H // NCORES  # heads per core
P = 128
G = 512  # q-group size
NG = T // G
KB = 128  # key block
NPO = C // P  # contraction chunks
F32 = mybir.dt.float32
F32R = mybir.dt.float32r
BF16 = mybir.dt.bfloat16

_nc_cache = {}


def _emit(tc):
    nc = tc.nc
    _last_pe = [None]
    xt = nc.dram_tensor("xt", [B, C, T], BF16, kind="ExternalInput").ap()
    wq2 = nc.dram_tensor("wq2", [P, NPO, 128], BF16, kind="ExternalInput").ap()
    wk2 = nc.dram_tensor("wk2", [P, NPO, 128], BF16, kind="ExternalInput").ap()
    wv2 = nc.dram_tensor("wv2", [P, NPO, 128], BF16, kind="ExternalInput").ap()
    wpt = nc.dram_tensor("wpt", [128, C], BF16, kind="ExternalInput").ap()
    tri = nc.dram_tensor("tri", [P, P], BF16, kind="ExternalInput").ap()
    identd = nc.dram_tensor("ident", [P, P], BF16, kind="ExternalInput").ap()
    out = nc.dram_tensor("out", [B, T, C], BF16, kind="ExternalOutput").ap()

    ctx = ExitStack()
    persist = ctx.enter_context(tc.tile_pool(name="persist", bufs=1))
    xt_pool = ctx.enter_context(tc.tile_pool(name="xtp", bufs=8))
    qk_pool = ctx.enter_context(tc.tile_pool(name="qkp", bufs=2))
    vt_pool = ctx.enter_context(tc.tile_pool(name="vtp", bufs=2))
    vaug_pool = ctx.enter_context(tc.tile_pool(name="vaugp", bufs=2))
    pt_pool = ctx.enter_context(tc.tile_pool(name="ptp", bufs=4))
    norm_pool = ctx.enter_context(tc.tile_pool(name="normp", bufs=2))
    ohat_pool = ctx.enter_context(tc.tile_pool(name="ohatp", bufs=2))
    out_pool = ctx.enter_context(tc.tile_pool(name="outp", bufs=6))
    st_psum = ctx.enter_context(tc.tile_pool(name="stps", bufs=1, space="PSUM"))
    ot_psum = ctx.enter_context(tc.tile_pool(name="otps", bufs=2, space="PSUM"))
    mm_psum = ctx.enter_context(tc.tile_pool(name="mmps", bufs=2, space="PSUM"))

    wq_sb = persist.tile([P, NPO, 128], BF16, tag="wq")
    wk_sb = persist.tile([P, NPO, 128], BF16, tag="wk")
    wv_sb = persist.tile([P, NPO, 128], BF16, tag="wv")
    wpt_sb = persist.tile([P, C], BF16, tag="wpt")
    tri_sb = persist.tile([P, P], BF16, tag="tri")
    ident = persist.tile([P, P], BF16, tag="ident")

    # ---- input loading: per-tg xT tiles, one 2MB DMA each ----
    def load_xt_tg(eng, b, tg, dep=None):
        t = xt_pool.tile([P, NPO, 512], BF16, tag="xt", name=f"xt{b}{tg}")
        i = eng.dma_start(
            t[:],
            xt[b, :, ts(tg, 512)].rearrange("(po pi) t -> pi po t", pi=P),
        )
        if dep is not None:
            add_dep_helper(i.ins, dep.ins, sync=True)
        return t, i

    # Fast ramp: tile0's po-chunks split across sync+scalar queues, with the
    # weight loads interleaved on scalar in the order the QKV groups need
    # them (wq before the first matmul, wk/wv before the K/V groups).
    # three queues in parallel at the head: scalar = weights only (wk/wv
    # land before the K/V groups need them), sync = even po-chunks + later
    # b0 tiles, gpsimd = odd po-chunks (idle until the b1 prefetch anyway)
    # wq in two pieces: the QKV po-chain's first matmuls start as soon as
    # the first two weight chunks land
    nc.scalar.dma_start(wq_sb[:, 0:2, :], wq2[:, 0:2, :])
    nc.scalar.dma_start(wq_sb[:, 2:, :], wq2[:, 2:, :])
    t00 = xt_pool.tile([P, NPO, 512], BF16, tag="xt", name="xt00")
    t00_last = None
    for po in range(NPO):
        eng = nc.sync if po % 2 == 0 else nc.gpsimd
        t00_last = eng.dma_start(t00[:, po, :], xt[0, ts(po, P), 0:512])
    nc.scalar.dma_start(wk_sb[:], wk2[:])
    nc.scalar.dma_start(wv_sb[:], wv2[:])
    nc.scalar.dma_start(tri_sb[:], tri[:])
    nc.scalar.dma_start(ident[:], identd[:])
    xt0, xt0_dmas = [t00], [t00_last]
    for tg in range(1, NG):
        eng = nc.sync if tg != 2 else nc.scalar
        t, i = load_xt_tg(eng, 0, tg, dep=xt0_dmas[tg - 2] if tg >= 2 else None)
        xt0.append(t)
        xt0_dmas.append(i)
        if tg == 1:
            nc.scalar.dma_start(wpt_sb[:], wpt[:])

    def new_state(b, xts):
        st = {
            "b": b,
            "xt": xts,
            "qt": qk_pool.tile([P, T], BF16, tag="qt", name=f"qt{b}"),
            "kt": qk_pool.tile([P, T], BF16, tag="kt", name=f"kt{b}"),
            "vt": vt_pool.tile([P, T], BF16, tag="vt", name=f"vt{b}"),
            "ohat": ohat_pool.tile([P, T], BF16, tag="ohat", name=f"oh{b}"),
            "vaug": [],
        }
        for h in range(HPC):
            va = vaug_pool.tile(
                [P, T // KB, 128], BF16, tag=f"vaug{h}", name=f"va{b}{h}"
            )
            nc.vector.memset(va[:, :, 64:128], 1.0)
            st["vaug"].append(va)
        return st

    def emit_qkv_group(st, which, tg):
        w_sb, dst = {
            "q": (wq_sb, st["qt"]),
            "k": (wk_sb, st["kt"]),
            "v": (wv_sb, st["vt"]),
        }[which]
        ps = mm_psum.tile([P, 512], F32, tag="mm", name=f"qkv{which}{tg}")
        for po in range(NPO):
            mi = nc.tensor.matmul(
                ps[:],
                w_sb[:, po, :],
                st["xt"][tg][:, po, :],
                start=(po == 0),
                stop=(po == NPO - 1),
            )
            _last_pe[0] = mi
        nc.vector.tensor_copy(dst[:, ts(tg, 512)], ps[:])

    def emit_vaug_part(st, kbg):
        vaug = st["vaug"]
        tps = mm_psum.tile([P, 4, P], BF16, tag="mm", name=f"vtr{kbg}")
        for kk in range(4):
            kb = 4 * kbg + kk
            nc.tensor.transpose(tps[:, kk, :], st["vt"][:, ts(kb, KB)], ident[:])
        for h in range(HPC):
            nc.vector.tensor_copy(
                vaug[h][:, 4 * kbg : 4 * kbg + 4, 0:64],
                tps[:, :, 64 * h : 64 * h + 64],
            )

    def emit_attn_g(st, g, filler=None):
        b, qt, kt, vaug, ohat = st["b"], st["qt"], st["kt"], st["vaug"], st["ohat"]
        l_sb = norm_pool.tile([P, G], F32, tag="lsb", name=f"l{b}{g}")
        rinv = norm_pool.tile([P, G], F32, tag="rinv", name=f"r{b}{g}")
        otps_h = [
            ot_psum.tile([P, G], F32, tag="ot", name=f"ot{h}") for h in range(HPC)
        ]
        n_j = 4 * g + 4
        for jg in range(math.ceil(n_j / 2)):
            js = [j for j in (2 * jg, 2 * jg + 1) if j < n_j]
            diag = 2 * jg >= 4 * g
            # per-j tiles holding BOTH heads: h0 -> [:,0,:] (PE rows 0:64,
            # bank A), h1 -> [:,1,:] (rows 64:128, bank B). The two S^T
            # matmuls issue back-to-back with disjoint row groups and
            # disjoint PSUM banks, so the PE runs them concurrently.
            stps_j = [
                st_psum.tile([P, 2, G], F32, tag=f"st{idx}", name=f"st{idx}")
                for idx in range(len(js))
            ]
            pt_j = [
                pt_pool.tile([P, 2, G], BF16, tag=f"pt{idx}", name=f"pt{idx}")
                for idx in range(len(js))
            ]
            for idx, j in enumerate(js):
                r = j - 4 * g
                q0 = 128 * r if r >= 0 else 0
                for h in range(HPC):
                    hb = 64 * h
                    nc.tensor.matmul(
                        stps_j[idx][:, h, q0:G],
                        kt[hb : hb + 64, ts(j, KB)],
                        qt[hb : hb + 64, G * g + q0 : G * (g + 1)],
                        start=True,
                        stop=True,
                    )
                nc.scalar.activation(
                    pt_j[idx][:, :, q0:G],
                    stps_j[idx][:, :, q0:G],
                    mybir.ActivationFunctionType.Exp,
                    scale=float(HS) ** -0.5,
                )
                if diag:
                    for h in range(HPC):
                        nc.gpsimd.tensor_tensor(
                            pt_j[idx][:, h, q0 : q0 + 128],
                            pt_j[idx][:, h, q0 : q0 + 128],
                            tri_sb[:],
                            mybir.AluOpType.mult,
                        )
            # independent filler work sits BETWEEN the S^T pair and the
            # exp-dependent O^T in the PE queue, covering the exp latency
            if filler is not None:
                filler()
            for idx, j in enumerate(js):
                r = j - 4 * g
                q0 = 128 * r if r >= 0 else 0
                for h in range(HPC):
                    nc.tensor.matmul(
                        otps_h[h][:, q0:G],
                        vaug[h][:, j, :],
                        pt_j[idx][:, h, q0:G],
                        start=(j == 0),
                        stop=(j == n_j - 1),
                    )
        # normalize straight out of PSUM: l rows copied (for a packed 128-wide
        # reciprocal), O rows consumed by tensor_tensor directly from PSUM
        for h in range(HPC):
            hb = 64 * h
            nc.vector.tensor_copy(l_sb[hb : hb + 64, :], otps_h[h][64:128, :])
        nc.vector.reciprocal_approx_fast(rinv[:], l_sb[:])
        for h in range(HPC):
            hb = 64 * h
            nc.vector.tensor_tensor(
                ohat[hb : hb + 64, ts(g, G)],
                otps_h[h][0:64, :],
                rinv[hb : hb + 64, :],
                mybir.AluOpType.mult,
            )

    _chunk_ctr = [0]

    def emit_proj_chunk(st, g, tc4, scalar_ok=True, dma_eng=None):
        b, ohat = st["b"], st["ohat"]
        t0 = G * g + P * tc4
        o_sb = out_pool.tile([P, C], BF16, tag="osb", name=f"osb{b}{g}{tc4}")
        for n in range(C // 512):
            pj = mm_psum.tile([P, 512], F32, tag="mm", name=f"pj{n}")
            nc.tensor.matmul(
                pj[:],
                ohat[:, t0 : t0 + P],
                wpt_sb[:, ts(n, 512)],
                start=True,
                stop=True,
            )
            # ~3/8 of the PSUM->SBUF copies go to ScalarE (except in the
            # scalar-bound drain phase), the rest to VectorE
            if scalar_ok and (2 * tc4 + n) % 8 in (1, 4, 6):
                nc.scalar.copy(o_sb[:, ts(n, 512)], pj[:])
            else:
                nc.vector.tensor_copy(o_sb[:, ts(n, 512)], pj[:])
        # alternate the two idle hardware queues (sync/gpsimd) for the 8MB
        # of output writes; all proj chunks execute after the xt prefetches
        # are done, so this doubles write throughput without starving inputs
        eng = dma_eng
        if eng is None:
            eng = nc.sync if _chunk_ctr[0] % 2 == 0 else nc.gpsimd
            _chunk_ctr[0] += 1
        eng.dma_start(out[b, t0 : t0 + P, :], o_sb[:])

    def emit_proj(st, g):
        for tc4 in range(G // P):
            emit_proj_chunk(st, g, tc4)

    # ================= pipelined emission =================
    st0 = new_state(0, xt0)
    st1 = new_state(1, [None] * NG)

    pending = []
    scalar_ok = [True]

    def filler():
        if pending:
            emit_proj_chunk(*pending.pop(0), scalar_ok=scalar_ok[0])

    # b1's attention groups g1/g2 interleave INTO the b0 loop as soon as
    # their QKV tiles exist, smoothing the scalar:PE ratio globally instead
    # of a PE-bound first half and a scalar-paced second half. b1's g0 (the
    # lightest group) runs last so the drain is cheap; proj chunks fill the
    # exp-latency slots of every late attention group.
    scalar_ok[0] = False
    xt1_dmas = []
    for tg in range(NG):
        for which in ("q", "k", "v"):
            emit_qkv_group(st0, which, tg)
        emit_vaug_part(st0, tg)
        emit_attn_g(st0, tg, filler=filler if tg == 3 else None)
        pending.extend((st0, tg, tc4) for tc4 in range(G // P))
        # b1 tiles on gpsimd's own hardware queue, gated behind b0's tiles
        # so they don't steal fabric bandwidth while b0 still needs it
        t, i = load_xt_tg(
            nc.gpsimd, 1, tg,
            dep=xt1_dmas[tg - 2] if tg >= 2 else xt0_dmas[NG - 1],
        )
        st1["xt"][tg] = t
        xt1_dmas.append(i)
        if tg >= 1:
            for which in ("q", "k", "v"):
                emit_qkv_group(st1, which, tg - 1)
        if tg == 2:
            emit_vaug_part(st1, 0)
            emit_vaug_part(st1, 1)
            emit_attn_g(st1, 1, filler=filler)
            pending.extend((st1, 1, tc4) for tc4 in range(G // P))
        if tg == 3:
            emit_vaug_part(st1, 2)
            emit_attn_g(st1, 2, filler=filler)
            pending.extend((st1, 2, tc4) for tc4 in range(G // P))
    for which in ("q", "k", "v"):
        emit_qkv_group(st1, which, NG - 1)
    emit_vaug_part(st1, 3)
    emit_attn_g(st1, 3, filler=filler)
    pending.extend((st1, 3, tc4) for tc4 in range(G // P))
    emit_attn_g(st1, 0, filler=lambda: (filler(), filler()))
    pending.extend((st1, 0, tc4) for tc4 in range(G // P))
    # final flush: ScalarE's exp work is done, so its DMA queue joins the
    # rotation -- three queues drain the trailing output writes
    qrot = [nc.sync, nc.gpsimd, nc.scalar]
    qi = 0
    while pending:
        emit_proj_chunk(*pending.pop(0), scalar_ok=False, dma_eng=qrot[qi % 3])
        qi += 1
    ctx.close()


def _build():
    if "nc" in _nc_cache:
        return _nc_cache["nc"]
    nc = bacc.Bacc("TRN2", target_bir_lowering=False, debug=False)
    with tile.TileContext(nc) as tc:
        _emit(tc)
    nc.compile()
    _nc_cache["nc"] = nc
    return nc


def _make_in_maps(x, wq, wk, wv, w_proj):
    import ml_dtypes

    bf16 = ml_dtypes.bfloat16
    xt = np.ascontiguousarray(x.transpose(0, 2, 1)).astype(bf16)
    tri = np.triu(np.ones((P, P), dtype=np.float32)).astype(bf16)
    ident = np.eye(P, dtype=np.float32).astype(bf16)
    def pack_w(w2):
        # [C, 128] -> [pi, po, d] so each partition's SBUF row is one
        # contiguous 2KB DMA line
        return np.ascontiguousarray(
            w2.reshape(NPO, P, 128).transpose(1, 0, 2)
        ).astype(bf16)

    in_maps = []
    for c in range(NCORES):
        h0 = HPC * c
        in_maps.append(
            {
                "xt": xt,
                "wq2": pack_w(
                    np.concatenate([wq[h0 + i] for i in range(HPC)], axis=1)
                ),
                "wk2": pack_w(
                    np.concatenate([wk[h0 + i] for i in range(HPC)], axis=1)
                ),
                "wv2": pack_w(
                    np.concatenate([wv[h0 + i] for i in range(HPC)], axis=1)
                ),
                "wpt": np.ascontiguousarray(
                    w_proj[:, 128 * c : 128 * (c + 1)].T
                ).astype(bf16),
                "tri": tri,
                "ident": ident,
            }
        )
    return in_maps


def kernel(x, wq, wk, wv, w_proj, b_proj):
    x = np.asarray(x, dtype=np.float32)
    wq = np.asarray(wq, dtype=np.float32)
    wk = np.asarray(wk, dtype=np.float32)
    wv = np.asarray(wv, dtype=np.float32)
    w_proj = np.asarray(w_proj, dtype=np.float32)
    b_proj = np.asarray(b_proj, dtype=np.float32)

    nc = _build()
    in_maps = _make_in_maps(x, wq, wk, wv, w_proj)
    res = run_bass_kernel_spmd(nc, in_maps, core_ids=list(range(NCORES)))
    acc = np.zeros((B, T, C), dtype=np.float64)
    for r in res.results:
        acc += np.asarray(r["out"], dtype=np.float32)
    return (acc + b_proj).astype(np.float32)

